# revision 13
# baseline (speedup 1.0000x reference)
"""AttentionHead kernel for 8 TRN2 NeuronCores.

Problem: q = x@Wq+bq; k = y@Wk+bk; v = y@Wv+bv
         att = softmax(q k^T / sqrt(128));  att = triu(att, k=1)  (AFTER softmax)
         out = att @ v
Shapes: x [4, 2048, 1024], y [4, 2048, 1024], W* [1024, 128], out [4, 2048, 128].

Sharding: 8 cores = (batch b in 0..3) x (query-half h in 0..1). Core (b, h)
computes queries [h*1024, (h+1)*1024) of batch b against all 2048 keys.
No cross-core communication.

SPMD uniformity trick: the post-softmax causal mask (keep key j > query i)
depends on the query offset h*1024, which differs per core, but all cores
must run the SAME graph. We rotate the key axis per core on host
(j_local = (j_global - h*1024) mod 2048). Then for every core:
  - keys j_local in [0, 1024): keep iff j_local > i_local  (same triangular
    band for every core -> one compile-time mask input shared by all cores)
  - keys j_local in [1024, 2048): keep-all for h=0, drop-all for h=1 ->
    handled by scaling those V tiles by a per-core scalar g in {1.0, 0.0}.
The softmax normalizer sums exp over ALL keys (mask comes after softmax),
and is invariant to the key rotation.

DMA strategy: the 16 SDMA engines round-robin between ACTIVE queues at
packet granularity, so a single queue gets the full ~400GB/s in strict
FIFO order.  All bf16 inputs are therefore host-packed into ONE dram
tensor ("stream") laid out per-partition-contiguous in exact consumption
order [Wk | y-kc0 | Wv | Wq | x-c0 | x-c1 | y-kc1 | y-kc2 | y-kc3 | tri],
and issued as 8 sequential dma_starts on the SP queue (boundaries = the
semaphore granularity consumers wait on).  Each dma lowers to 128
contiguous 2-8KB descriptors.  consts (f32) rides the GPSIMD queue; the
ACT queue only carries the mid-kernel output store.

The compute phase order follows the supply order: warm-up -> kproj kc0 ->
vproj kc0 -> transposes -> qproj c0 -> scores.  Garbage warm-up matmuls
(a) release the HAM clock gate (PE runs 1.2GHz until ~3.4us of sustained
busy) and (b) bridge every known supply wait so the PE never idles (an
idle >3.4us re-throttles the clock to 1.2GHz).

On-chip layout: host pre-transposes x/y to [feature, seq] bf16 so the
projections produce qT [d, i] / kT [d, j] / vT [d, j] directly in the
layouts the PE array needs; vT is PE-transposed to v [j, d] tiles.
Scores are computed transposed, ST [j, i].  The normalizer Z sums exp
over tiles t0..t13 in ONE bf16 DVE accumulator per chunk; Z = 3
ones-matmuls per chunk (accumulator + t14 + t15 straight from exp, so the
tail has no DVE add).  O^T [d, i] = sum_j v[j, d]^T . maskedexp[j, i] is
stored UNNORMALIZED in bf16 together with Z [1, 1024] f32; the host does
out = (O^T / Z)^T.  This removes reciprocal+multiply from the device tail.

The k/v projections, V transposes and the attention t-loop are fused
along key chunks of 512 so ACT's exp stream overlaps the projection
matmuls instead of running after them.
"""

import numpy as np
import ml_dtypes

B = 4
LQ = 2048
LK = 2048
XS = 1024
PD = 128
LQS = LQ // 2  # queries per core: 1024

NE = XS // 128  # 8 contraction tiles for projections
NT = LK // 128  # 16 key tiles
CH = 512  # chunk (PSUM bank = 512 f32)
NCH = LQS // CH  # 2 query chunks
NKC = LK // CH  # 4 key chunks
SM_SCALE = 1.0 / float(np.sqrt(PD))

WARM_N = 8  # warm-up matmuls (cold ~427ns each; bridge DMA + HAM window)
FILL_MID = 2  # garbage matmuls between kproj e0-3 and e4-7
FILL_V = 1  # garbage matmuls before vproj
FILL_Q = 2  # garbage matmuls before qproj c0
FILL_Q1 = 1  # garbage matmuls before qproj c1

# stream column offsets (bf16 elements per partition)
OFF_WK = 0
OFF_Y0 = OFF_WK + XS
OFF_WV = OFF_Y0 + NE * CH
OFF_WQ = OFF_WV + XS
OFF_X0 = OFF_WQ + XS
OFF_X1 = OFF_X0 + NE * CH
OFF_Y1 = OFF_X1 + NE * CH
OFF_Y2 = OFF_Y1 + NE * CH
OFF_Y3 = OFF_Y2 + NE * CH
OFF_TRI = OFF_Y3 + NE * CH  # mask graph only

_BF16 = ml_dtypes.bfloat16

_graph_cache = {}


def _build_graph(apply_mask: bool):
    import concourse.mybir as mybir
    from concourse import bacc
    from concourse.masks import make_identity
    from concourse.tile import TileContext

    BF = mybir.dt.bfloat16
    F32 = mybir.dt.float32
    Exp = mybir.ActivationFunctionType.Exp
    Identity = mybir.ActivationFunctionType.Identity

    NCOLS = OFF_TRI + (896 if apply_mask else 0)

    nc = bacc.Bacc()

    stream = nc.declare_dram_parameter("stream", [128, NCOLS], BF, isOutput=False)
    # cols 0..2 = bq, bk, bv; cols 3..18 = per-v-tile scale g.
    consts = nc.declare_dram_parameter("consts", [128, 3 + NT], F32, isOutput=False)
    out_ext = nc.declare_dram_parameter("out", [PD, LQS], BF, isOutput=True)
    z_ext = nc.declare_dram_parameter("zout", [1, LQS], F32, isOutput=True)

    with TileContext(nc) as tc:
        with (
            tc.tile_pool(name="const", bufs=1) as const_pool,
            tc.tile_pool(name="sb", bufs=1) as sb_pool,
            tc.tile_pool(name="exp", bufs=3) as exp_pool,
            tc.tile_pool(name="ps", bufs=2, space="PSUM") as ps_pool,
            tc.tile_pool(name="psacc", bufs=1, space="PSUM") as psacc_pool,
        ):
            sin = sb_pool.tile([128, NCOLS], BF)
            consts_sb = const_pool.tile([128, 3 + NT], F32)

            # ---- input DMAs: one queue, strict consumption order; each
            # dma_start boundary is a consumer-visible semaphore.
            def sdma(lo, hi):
                nc.sync.dma_start(out=sin[:, lo:hi], in_=stream[:, lo:hi])

            sdma(OFF_WK, OFF_Y0 + 2 * CH)  # Wk + y-kc0 e0-1
            sdma(OFF_Y0 + 2 * CH, OFF_Y0 + 4 * CH)  # y-kc0 e2-3
            sdma(OFF_Y0 + 4 * CH, OFF_Y0 + 6 * CH)  # y-kc0 e4-5
            sdma(OFF_Y0 + 6 * CH, OFF_WV)  # y-kc0 e6-7
            sdma(OFF_WV, OFF_X0)  # Wv + Wq
            sdma(OFF_X0, OFF_X0 + 4 * CH)  # x c0 e0-3
            sdma(OFF_X0 + 4 * CH, OFF_X1)  # x c0 e4-7
            sdma(OFF_X1, OFF_Y1)  # x c1
            sdma(OFF_Y1, OFF_Y2)  # y kc1
            sdma(OFF_Y2, OFF_Y3)  # y kc2
            sdma(OFF_Y3, NCOLS)  # y kc3 (+ tri)

            def Wk_e(e):
                return sin[:, OFF_WK + e * 128:OFF_WK + (e + 1) * 128]

            def Wv_e(e):
                return sin[:, OFF_WV + e * 128:OFF_WV + (e + 1) * 128]

            def Wq_e(e):
                return sin[:, OFF_WQ + e * 128:OFF_WQ + (e + 1) * 128]

            def y_ap(kc, e):
                off = [OFF_Y0, OFF_Y1, OFF_Y2, OFF_Y3][kc] + e * CH
                return sin[:, off:off + CH]

            def x_ap(c, e):
                off = [OFF_X0, OFF_X1][c] + e * CH
                return sin[:, off:off + CH]

            tri_sb = sin[:, OFF_TRI:OFF_TRI + 896] if apply_mask else None
            bq_sb = consts_sb[:, 0:1]
            bk_sb = consts_sb[:, 1:2]
            bv_sb = consts_sb[:, 2:3]
            gv_sb = consts_sb[:, 3:]

            # ---- constants, identity, ACT table prime.  The memsets run
            # on GpSimd (its sequencer reaches "main" ~1.2us before DVE's)
            # so the PE warm-up can start earlier; consts + identity follow
            # on the same engine, well before they are needed.
            ones_sb = const_pool.tile([128, 128], BF)
            warm_rhs = const_pool.tile([128, CH], BF)
            nc.gpsimd.memset(warm_rhs, 1.0)
            nc.gpsimd.memset(ones_sb, 1.0)
            nc.gpsimd.dma_start(out=consts_sb, in_=consts[:, :])
            ident_sb = const_pool.tile([128, 128], BF)
            make_identity(nc, ident_sb)
            # Touch Exp early so the ~1.3us ACT_TABLE_LOAD overlaps the DMAs.
            scratch1 = const_pool.tile([1, 1], F32)
            nc.scalar.activation(scratch1, ones_sb[0:1, 0:1], Exp)
            warm_ps = ps_pool.tile(
                [128, CH], mybir.dt.float32, tag="rot", bufs=5, name="warm"
            )
            # transposes get their own 4-deep PSUM slots (half a bank) so
            # score matmuls never inherit a WAR dependency on the DVE
            # v-scale drains through the rotating pool
            pst_ps = psacc_pool.tile([128, 8, PD], BF, tag="pst")

            def emit_warm(n):
                for _ in range(n):
                    nc.tensor.matmul(warm_ps, lhsT=ones_sb, rhs=warm_rhs,
                                     start=True, stop=True)

            emit_warm(WARM_N)

            kT_sb = sb_pool.tile([128, LK], BF)
            vT_sb = sb_pool.tile([128, LK], BF)
            v_sb = sb_pool.tile([128, NT, PD], BF)
            qT_sb = sb_pool.tile([128, LQS], BF)
            z_ps = [None, None]
            o_ps = [
                psacc_pool.tile(
                    [128, CH], mybir.dt.float32, tag=f"o{c}", name=f"o_ps{c}"
                )
                for c in range(NCH)
            ]

            def emit_qproj(c):
                cs = slice(c * CH, (c + 1) * CH)
                ps = ps_pool.tile(
                    [128, CH], mybir.dt.float32, tag="rot", bufs=5, name="qps"
                )
                for e in range(NE):
                    nc.tensor.matmul(
                        ps,
                        lhsT=Wq_e(e),
                        rhs=x_ap(c, e),
                        start=(e == 0),
                        stop=(e == NE - 1),
                    )
                nc.vector.tensor_scalar_add(qT_sb[:, cs], ps, bq_sb)

            def emit_ktproj(kc, mid_fill=0):
                ks = slice(kc * CH, (kc + 1) * CH)
                ps = ps_pool.tile(
                    [128, CH], mybir.dt.float32, tag="rot", bufs=5, name="kps"
                )
                for e in range(NE):
                    if e == 4 and mid_fill:
                        emit_warm(mid_fill)
                    nc.tensor.matmul(
                        ps,
                        lhsT=Wk_e(e),
                        rhs=y_ap(kc, e),
                        start=(e == 0),
                        stop=(e == NE - 1),
                    )
                nc.scalar.activation(kT_sb[:, ks], ps, Identity, bias=bk_sb)

            def make_kt_fillers(kc):
                ks = slice(kc * CH, (kc + 1) * CH)
                ps = ps_pool.tile(
                    [128, CH], mybir.dt.float32, tag="rot", bufs=5, name="kps"
                )

                def step(e):
                    nc.tensor.matmul(
                        ps,
                        lhsT=Wk_e(e),
                        rhs=y_ap(kc, e),
                        start=(e == 0),
                        stop=(e == NE - 1),
                    )
                    if e == NE - 1:
                        nc.scalar.activation(kT_sb[:, ks], ps, Identity, bias=bk_sb)

                return [lambda e=e: step(e) for e in range(NE)]

            def make_v_fillers(kc):
                ks = slice(kc * CH, (kc + 1) * CH)
                ps = ps_pool.tile(
                    [128, CH], mybir.dt.float32, tag="rot", bufs=5, name="vps"
                )

                def step(e):
                    nc.tensor.matmul(
                        ps,
                        lhsT=Wv_e(e),
                        rhs=y_ap(kc, e),
                        start=(e == 0),
                        stop=(e == NE - 1),
                    )
                    if e == NE - 1:
                        nc.scalar.activation(vT_sb[:, ks], ps, Identity, bias=bv_sb)

                def trstep(t):
                    pst = pst_ps[:, t % 8, :]
                    nc.tensor.transpose(
                        pst, vT_sb[:, t * 128:(t + 1) * 128], ident_sb
                    )
                    nc.vector.tensor_scalar_mul(v_sb[:, t, :], pst, gv_sb[:, t:t + 1])

                return [lambda e=e: step(e) for e in range(NE)] + [
                    lambda t=t: trstep(t) for t in range(4 * kc, 4 * kc + 4)
                ]

            ek_acc = [
                sb_pool.tile([128, CH], BF, name=f"ek_acc{c}") for c in range(NCH)
            ]
            e_tiles = {}

            def emit_st_exp(t, c):
                ts_ = slice(t * 128, (t + 1) * 128)
                cs = slice(c * CH, (c + 1) * CH)
                st = ps_pool.tile(
                    [128, CH], mybir.dt.float32, tag="rot", bufs=5, name="st"
                )
                # ST [j, i] = kT_t^T qT (full d contraction in one shot)
                nc.tensor.matmul(
                    st, lhsT=kT_sb[:, ts_], rhs=qT_sb[:, cs], start=True, stop=True
                )
                e_sb = exp_pool.tile([128, CH], BF, bufs=12, name="e_sb")
                nc.scalar.activation(e_sb, st, Exp, scale=SM_SCALE)
                e_tiles[t, c] = e_sb
                # running unmasked sum of tiles t0..t13 for the normalizer;
                # t14/t15 go straight into Z matmuls (no DVE add in the tail)
                if t == 0:
                    nc.vector.tensor_copy(ek_acc[c], e_sb)
                elif t < NT - 2:
                    nc.vector.tensor_add(ek_acc[c], ek_acc[c], e_sb)
                if apply_mask and t < 8 and t // 4 == c:
                    # band tile: columns >= (t%4+1)*128 are fully masked, so
                    # the multiply (and the matching P5 matmul) can shrink to
                    # the live width -- except for the group-start tile, which
                    # stays full-width so start=True covers the whole bank.
                    w = _band_w(t)
                    off = 384 - (128 * t - CH * c)
                    nc.vector.tensor_mul(
                        e_sb[:, 0:w], e_sb[:, 0:w], tri_sb[:, off:off + w]
                    )

            def _band_w(t):
                if t % 4 in (1, 2):
                    return (t % 4 + 1) * 128
                return CH

            def emit_p5(t, c):
                # O^T [d, i] += v_t^T @ maskedexp ; skip all-zero tiles
                if (not apply_mask) or t >= 4 * c:
                    first_t = 4 * c if apply_mask else 0
                    w = _band_w(t) if (apply_mask and t < 8 and t // 4 == c) else CH
                    nc.tensor.matmul(
                        o_ps[c][:, 0:w],
                        lhsT=v_sb[:, t, :],
                        rhs=e_tiles[t, c][:, 0:w],
                        start=(t == first_t),
                        stop=(t == NT - 1),
                    )

            out_sb = sb_pool.tile([128, LQS], BF)
            z_sb = sb_pool.tile([128, LQS], F32)

            def emit_finalize(c):
                cs = slice(c * CH, (c + 1) * CH)
                # unnormalized O^T out in bf16; Z row out in f32 (host
                # divides).  O copy on DVE, Z copy on ACT -- parallel drains.
                nc.vector.tensor_copy(out_sb[:, cs], o_ps[c])
                nc.scalar.activation(z_sb[:, cs], z_ps[c], Identity)
                eng = nc.scalar if c == 0 else nc.sync
                eng.dma_start(out=out_ext[:, cs], in_=out_sb[:, cs])
                if c == NCH - 1:
                    nc.scalar.dma_start(out=z_ext[0:1, :], in_=z_sb[0:1, :])

            # ---- supply-ordered prologue
            emit_ktproj(0, mid_fill=FILL_MID)
            emit_warm(FILL_V)
            fillers0 = make_v_fillers(0)
            for f in fillers0[:NE]:  # vproj kc0
                f()
            emit_warm(FILL_Q)
            for f in fillers0[NE:]:  # transposes t0-3
                f()
            emit_qproj(0)

            # chunk 0 phase: scores for both query chunks over key tiles 0-3,
            # P5s pipelined behind; qproj c1 between the two chunks (its DMA
            # lands just in time).
            for t in range(4):
                emit_st_exp(t, 0)
            emit_warm(FILL_Q1)
            emit_qproj(1)
            pend = []
            for t in range(4):
                emit_st_exp(t, 1)
                if len(pend) >= 2:
                    emit_p5(*pend.pop(0))
                    emit_p5(*pend.pop(0))
                pend.append((t, 0))
                pend.append((t, 1))
            for tc_ in pend:
                emit_p5(*tc_)
            for f in make_kt_fillers(1) + make_v_fillers(1):
                f()

            # steady phases: scores + inline P5s, fillers = next chunk's
            # kT/vT/transposes spread over the score slots
            for kc in range(1, NKC - 1):
                fillers = make_kt_fillers(kc + 1) + make_v_fillers(kc + 1)
                nfill = len(fillers)
                fi = 0
                pend = []
                pairs = [(t, c) for t in range(4 * kc, 4 * kc + 4) for c in range(NCH)]
                for i, (t, c) in enumerate(pairs):
                    emit_st_exp(t, c)
                    # emit P5s in same-t pairs so the v_t stationary is
                    # loaded once for both query chunks
                    if len(pend) >= 4:
                        emit_p5(*pend.pop(0))
                        emit_p5(*pend.pop(0))
                    pend.append((t, c))
                    # thread next-chunk fillers into the later score slots
                    # only -- their yT chunk is still in flight during the
                    # early slots of this phase (in-order PE queue would
                    # head-of-line block)
                    while fi < max(0, i - 2) * nfill // 5:
                        fillers[fi]()
                        fi += 1
                for tc_ in pend:
                    emit_p5(*tc_)

            # last chunk runs query-chunk-major (no fillers left, so no
            # head-of-line risk) so chunk 0's Z + store overlap chunk 1's
            # scores
            kc = NKC - 1

            def emit_z(c, which, start, stop):
                nc.tensor.matmul(
                    z_ps[c], lhsT=ones_sb, rhs=which, start=start, stop=stop
                )

            # chunk 0 block, with chunk 1's first scores interleaved so c1's
            # exp/ek pipeline is already primed when its own block starts
            z_ps[0] = ps_pool.tile(
                [128, CH], mybir.dt.float32, tag="rot", bufs=5, name="z0"
            )
            for t in range(4 * kc, 4 * kc + 4):
                emit_st_exp(t, 0)
            emit_p5(4 * kc, 0)
            emit_p5(4 * kc + 1, 0)
            emit_st_exp(4 * kc, 1)
            emit_st_exp(4 * kc + 1, 1)
            # Z: bf16 running sum of t0..t13, then t14 and t15 straight
            # from their exp tiles
            emit_z(0, ek_acc[0], True, False)
            emit_p5(4 * kc + 2, 0)
            emit_z(0, e_tiles[NT - 2, 0], False, False)
            emit_p5(NT - 1, 0)
            emit_z(0, e_tiles[NT - 1, 0], False, True)
            emit_finalize(0)

            z_ps[1] = ps_pool.tile(
                [128, CH], mybir.dt.float32, tag="rot", bufs=5, name="z1"
            )
            emit_st_exp(4 * kc + 2, 1)
            emit_st_exp(4 * kc + 3, 1)
            emit_p5(4 * kc, 1)
            emit_p5(4 * kc + 1, 1)
            emit_z(1, ek_acc[1], True, False)
            emit_p5(4 * kc + 2, 1)
            emit_z(1, e_tiles[NT - 2, 1], False, False)
            emit_p5(NT - 1, 1)
            emit_z(1, e_tiles[NT - 1, 1], False, True)
            emit_finalize(1)

    nc.finalize()
    return nc


def _get_graph(apply_mask: bool):
    key = bool(apply_mask)
    if key not in _graph_cache:
        _graph_cache[key] = _build_graph(key)
    return _graph_cache[key]


def kernel(**inputs) -> np.ndarray:
    from concourse.bass_utils import run_bass_kernel_spmd

    x = np.asarray(inputs["x"], dtype=np.float32)
    y = np.asarray(inputs["y"], dtype=np.float32)
    Wq = np.asarray(inputs["Wq"], dtype=np.float32)
    Wk = np.asarray(inputs["Wk"], dtype=np.float32)
    Wv = np.asarray(inputs["Wv"], dtype=np.float32)
    bq = np.asarray(inputs["bq"], dtype=np.float32)
    bk = np.asarray(inputs["bk"], dtype=np.float32)
    bv = np.asarray(inputs["bv"], dtype=np.float32)
    mask = bool(np.asarray(inputs["mask"]).item())

    nc = _get_graph(mask)

    def pack_w(W):
        # [1024 f, 128 d] -> [128 p, 8 e, 128 d] -> [128, 1024]
        return W.reshape(NE, 128, PD).transpose(1, 0, 2).reshape(128, NE * PD)

    Wk_p, Wv_p, Wq_p = pack_w(Wk), pack_w(Wv), pack_w(Wq)
    if mask:
        cc = np.arange(896, dtype=np.int64)[None, :] - 384
        jj = np.arange(128, dtype=np.int64)[:, None]
        tri = (jj > cc).astype(np.float32)

    in_maps = []
    for core in range(8):
        b, h = core // 2, core % 2
        qoff = h * LQS
        xs = x[b, qoff:qoff + LQS, :]
        ys = np.roll(y[b], -qoff, axis=0) if qoff else y[b]
        g = 1.0 if (h == 0 or not mask) else 0.0
        consts_arr = np.ones((128, 3 + NT), dtype=np.float32)
        consts_arr[:, 0] = bq
        consts_arr[:, 1] = bk
        consts_arr[:, 2] = bv
        consts_arr[:, 3 + NT // 2:] = g
        # [f, i] -> [8 e, 128 p, nch c, 512 ii] -> [p, c, e*512+ii]
        xT4 = xs.T.reshape(NE, 128, NCH, CH).transpose(1, 2, 0, 3).reshape(
            128, NCH, NE * CH
        )
        yT4 = ys.T.reshape(NE, 128, NKC, CH).transpose(1, 2, 0, 3).reshape(
            128, NKC, NE * CH
        )
        parts = [
            Wk_p, yT4[:, 0], Wv_p, Wq_p, xT4[:, 0], xT4[:, 1],
            yT4[:, 1], yT4[:, 2], yT4[:, 3],
        ]
        if mask:
            parts.append(tri)
        strm = np.ascontiguousarray(np.concatenate(parts, axis=1)).astype(_BF16)
        m = {
            "stream": strm,
            "consts": consts_arr,
        }
        in_maps.append(m)

    res = run_bass_kernel_spmd(nc, in_maps, core_ids=list(range(8)))

    out = np.empty((B, LQ, PD), dtype=np.float32)
    for core in range(8):
        b, h = core // 2, core % 2
        qoff = h * LQS
        ot = res.results[core]["out"].astype(np.float32)  # [128 d, 1024 i]
        z = np.asarray(res.results[core]["zout"], dtype=np.float32)  # [1, 1024]
        out[b, qoff:qoff + LQS, :] = (ot / z).T
    return out
